# revision 54
# baseline (speedup 1.0000x reference)
"""Two-layer GAT on 8 Trainium2 NeuronCores (Bass/Tile).

Strategy (graph/data parallel, per sharding hint):
  - Nodes sharded by destination across 8 cores (slab = N/8 each).
  - Per layer, per edge we gather from HBM tables:
      feat-table  [N, 256] bf16  (pre-transformed features x@W1 / relu(h1)@W2)
      attn-table  [N, 64] f32    (a_src in cols 0:4; 256B rows for dma_gather)
      dst-table   [SLAB, 64] f32 (a_dst of own nodes, local dst index)
  - Edges grouped into 128-aligned destination node ranges (SPMD-uniform
    structure), each range's edges split into A (src < 32768) / B groups for
    int16 gather indices, padded to 128-slot sub-blocks.
  - Per 128-edge sub-block: host-provided static fp8 one-hot S / S^T
    matrices (DMA-streamed); TensorE matmuls broadcast a_dst to edges (S^T)
    and reduce softmax-weighted messages + z into PSUM node-major
    accumulators. No segment max (exp-safe value range). Gathers ride 4
    SWDGE queues; attention/messages/aggregation are streamed per gather
    group behind the gathers.
  - Layer 1 epilogue computes relu(h1) and immediately the layer-2 table row
    chunk (transpose + W2_ext matmul); slabs are AllGather'd between layers.
"""

import numpy as np
import ml_dtypes
from contextlib import ExitStack

import concourse.bass as bass
import concourse.tile as tile
import concourse.bacc as bacc
from concourse import mybir
from concourse.bass_utils import run_bass_kernel_spmd

bf16 = ml_dtypes.bfloat16
F32 = mybir.dt.float32
BF16 = mybir.dt.bfloat16
I16 = mybir.dt.int16
FP8 = mybir.dt.float8e4
fp8 = ml_dtypes.float8_e4m3

NCORES = 8
HEADS = 4
TRACE = False
LAST_RESULT = None
NEG = 0.2
RANGES_PER_MACRO = 2
SKIP_L2PREP = False
AGG_CUT = 'full'


# ---------------------------------------------------------------- host prep

def _fold_ext(W, att_src, att_dst):
    H, C = att_src.shape
    Ws = (W.reshape(W.shape[0], H, C) * att_src[None]).sum(-1)
    Wd = (W.reshape(W.shape[0], H, C) * att_dst[None]).sum(-1)
    return np.concatenate([W, Ws, Wd], axis=1)  # [Cin, Cout + 2H]


def _wrap16(idx):
    """int16 idx list (len % 16 == 0) -> [128, len/16] wrapped+replicated."""
    w = np.asarray(idx, np.int16).reshape(-1, 16).T  # [16, len/16]
    return np.tile(w, (8, 1))


class Meta:
    pass


def _build_structure(src, dst, N, slab, grp_of, idx_of, self_direct=False):
    """Uniform (cross-core) macro/range/sub-block structure + per-core arrays.

    grp_of(s) -> bool array (False=table A, True=table B);
    idx_of(s) -> int16-range row index within that table."""
    n_ranges = -(-slab // 128)
    per_core = []
    for core in range(NCORES):
        lo = core * slab
        m = (dst >= lo) & (dst < lo + slab)
        s = src[m].astype(np.int64)
        d = (dst[m] - lo).astype(np.int64)
        order = np.argsort(d, kind="stable")
        s, d = s[order], d[order]
        starts = np.concatenate([[0], np.cumsum(np.bincount(d, minlength=slab))])
        per_core.append((s, d, starts))

    # per-range A/B slot budgets: max over cores, rounded up to 128
    A_r = np.zeros(n_ranges, np.int64)
    B_r = np.zeros(n_ranges, np.int64)
    for core in range(NCORES):
        s, d, starts = per_core[core]
        for r in range(n_ranges):
            e0, e1 = starts[r * 128], starts[min((r + 1) * 128, slab)]
            es, ed = s[e0:e1], d[e0:e1]
            if self_direct:
                ns = es != ed + core * slab
                es = es[ns]
            a = int((~grp_of(es)).sum())
            A_r[r] = max(A_r[r], a)
            B_r[r] = max(B_r[r], len(es) - a)
    A_r = (-(-A_r // 128) * 128).astype(np.int64)
    B_r = (-(-B_r // 128) * 128).astype(np.int64)

    # macros: groups of ranges
    macros = []
    for m0 in range(0, n_ranges, RANGES_PER_MACRO):
        rs = list(range(m0, min(m0 + RANGES_PER_MACRO, n_ranges)))
        mm = Meta()
        mm.ranges = rs
        mm.nA = int(sum(A_r[r] for r in rs)) // 128   # A sub-blocks
        mm.nB = int(sum(B_r[r] for r in rs)) // 128
        mm.nblk = mm.nA + mm.nB + (len(rs) if self_direct else 0)
        blocks = []
        for i, r in enumerate(rs):
            for _ in range(A_r[r] // 128):
                blocks.append([i, False, False])
        for i, r in enumerate(rs):
            for _ in range(B_r[r] // 128):
                blocks.append([i, False, False])
        if self_direct:
            mm.selfs = [(mm.nA + mm.nB + i, i) for i in range(len(rs))]
            for i, r in enumerate(rs):
                blocks.append([i, False, False])
        else:
            mm.selfs = []
        seen = set()
        for b in blocks:
            if b[0] not in seen:
                b[1] = True
                seen.add(b[0])
        last = {}
        for k, b in enumerate(blocks):
            last[b[0]] = k
        for i, k in last.items():
            blocks[k][2] = True
        mm.blocks = blocks
        mm.n_nodes = [min(128, slab - r * 128) for r in rs]
        mm.node_base = [r * 128 for r in rs]
        macros.append(mm)

    col_i = 0
    col_o = 0
    for mm in macros:
        mm.ci_A = col_i
        col_i += (mm.nA * 128) // 16
        mm.ci_B = col_i
        col_i += (mm.nB * 128) // 16
        mm.ci_D = col_i
        col_i += (mm.nblk * 128) // 16
        mm.co = col_o
        col_o += mm.nblk
    WI, WO = col_i, col_o

    IDXs, SFs, STFs = [], [], []
    rng128 = np.arange(128)
    for core in range(NCORES):
        s, d, starts = per_core[core]
        IDX = np.zeros((128, WI), np.int16)
        SFa = np.zeros((128, WO * 128), fp8)
        STFa = np.zeros((128, WO * 128), fp8)
        for mm in macros:
            idxA, idxB, oA, oB, dA, dB = [], [], [], [], [], []
            oS = []
            for i, r in enumerate(mm.ranges):
                e0, e1 = starts[r * 128], starts[min((r + 1) * 128, slab)]
                es, ed = s[e0:e1], d[e0:e1]
                if self_direct:
                    ns = es != ed + core * slab
                    es, ed = es[ns], ed[ns]
                    w = min(r * 128, slab - 128)
                    sh = r * 128 - w
                    oS += [(e - sh if e >= sh and (w + e) < slab else -1)
                           for e in range(128)]
                selA = ~grp_of(es)
                padA = int(A_r[r] - selA.sum())
                padB = int(B_r[r] - (~selA).sum())
                idxA += list(idx_of(es[selA])) + [0] * padA
                oA += list(ed[selA] - r * 128) + [-1] * padA
                dA += list(ed[selA]) + [0] * padA
                idxB += list(idx_of(es[~selA])) + [0] * padB
                oB += list(ed[~selA] - r * 128) + [-1] * padB
                dB += list(ed[~selA]) + [0] * padB
            if mm.nA:
                IDX[:, mm.ci_A:mm.ci_B] = _wrap16(idxA)
            if mm.nB:
                IDX[:, mm.ci_B:mm.ci_D] = _wrap16(idxB)
            IDX[:, mm.ci_D:mm.ci_D +
                ((mm.nA + mm.nB) * 128) // 16] = _wrap16(dA + dB)
            oo = np.array(oA + oB + oS, np.int32).reshape(mm.nblk, 128)
            oh = (oo[:, :, None] == rng128[None, None, :])  # [j, e, n]
            SFa[:, mm.co * 128:(mm.co + mm.nblk) * 128] = \
                oh.transpose(1, 0, 2).reshape(128, -1).astype(fp8)
            STFa[:, mm.co * 128:(mm.co + mm.nblk) * 128] = \
                oh.transpose(2, 0, 1).reshape(128, -1).astype(fp8)
        IDXs.append(IDX)
        SFs.append(SFa)
        STFs.append(STFa)

    st = Meta()
    st.n_ranges = n_ranges
    st.macros = macros
    st.WI, st.WO = WI, WO
    st.IDXs, st.SFs, st.STFs = IDXs, SFs, STFs
    return st


# ---------------------------------------------------------------- kernel build

def _mid_bcast(ap, count, pos):
    """Insert a [0, count] dim at position `pos` of an AP's ap list."""
    a = ap.ap.copy()
    a.insert(pos, [0, count])
    return bass.AP(tensor=ap.tensor, offset=ap.offset, ap=a)


def _build_nc(st1, st2, N, slab, C1, C2, Cin, stage=4):
    nc = bacc.Bacc("TRN2", target_bir_lowering=False, debug=False,
                   num_devices=NCORES, num_swdge_queues=4)
    EXT1 = C1 + 2 * HEADS
    EXT2 = C2 + 2 * HEADS
    FW1 = C1 + 128          # folded feat+attn table row (bf16 cols, 256B mult)
    FW2 = C2 + 128
    n_t1 = -(-N // 128)
    n_tr = st1.n_ranges
    split = 32768
    CH0 = 3328               # layer-2 AllGather chunk boundary (local rows)
    CH1 = slab - CH0

    xT = nc.dram_tensor("xT", [Cin, N], BF16, kind="ExternalInput")
    xoT = nc.dram_tensor("xoT", [Cin, slab], BF16, kind="ExternalInput")
    w1e = nc.dram_tensor("w1e", [Cin, EXT1], BF16, kind="ExternalInput")
    w2e = nc.dram_tensor("w2e", [128, C1 // 128, EXT2], BF16, kind="ExternalInput")
    b1 = nc.dram_tensor("b1", [1, C1], F32, kind="ExternalInput")
    b2 = nc.dram_tensor("b2", [1, C2], F32, kind="ExternalInput")
    idf32 = nc.dram_tensor("idf32", [128, 128], F32, kind="ExternalInput")
    IDX1 = nc.dram_tensor("IDX1", [128, st1.WI], I16, kind="ExternalInput")
    SF1 = nc.dram_tensor("SF1", [128, st1.WO * 128], FP8, kind="ExternalInput")
    STF1 = nc.dram_tensor("STF1", [128, st1.WO * 128], FP8,
                          kind="ExternalInput")
    IDX2 = nc.dram_tensor("IDX2", [128, st2.WI], I16, kind="ExternalInput")
    SF2 = nc.dram_tensor("SF2", [128, st2.WO * 128], FP8, kind="ExternalInput")
    STF2 = nc.dram_tensor("STF2", [128, st2.WO * 128], FP8,
                          kind="ExternalInput")

    t1tab = nc.dram_tensor("t1tab", [N, FW1], BF16)
    d1attn = nc.dram_tensor("d1attn", [slab, 4], F32)
    agf_in = nc.dram_tensor("agf_in", [slab, FW2], BF16)
    t2a = nc.dram_tensor("t2a", [NCORES * CH0, FW2], BF16, addr_space="Shared")
    t2b = nc.dram_tensor("t2b", [NCORES * CH1, FW2], BF16, addr_space="Shared")
    d2attn = nc.dram_tensor("d2attn", [slab, 4], F32)
    out = nc.dram_tensor("out", [slab, C2], F32, kind="ExternalOutput")

    reg_cache = {}

    def nreg(v):
        if v not in reg_cache:
            reg_cache[v] = nc.gpsimd.to_reg(v)
        return reg_cache[v]

    def rows_write(eng, dram, row0, total_rows, sb_tile, width, ntiles):
        full = total_rows // 128
        rem = total_rows - full * 128
        if full:
            eng.dma_start(
                bass.AP(tensor=dram.ap().tensor, offset=row0 * width,
                        ap=[[width, 128], [128 * width, full], [1, width]]),
                sb_tile[:, 0:full, :])
        if rem:
            eng.dma_start(
                bass.AP(tensor=dram.ap().tensor,
                        offset=(row0 + full * 128) * width,
                        ap=[[width, rem], [1, width]]),
                sb_tile[:rem, full, :])

    def rows_read(eng, dram, row0, total_rows, sb_tile, width):
        full = total_rows // 128
        rem = total_rows - full * 128
        if full:
            eng.dma_start(
                sb_tile[:, 0:full, :],
                bass.AP(tensor=dram.ap().tensor, offset=row0 * width,
                        ap=[[width, 128], [128 * width, full], [1, width]]))
        if rem:
            eng.dma_start(
                sb_tile[:rem, full, :],
                bass.AP(tensor=dram.ap().tensor,
                        offset=(row0 + full * 128) * width,
                        ap=[[width, rem], [1, width]]))

    with tile.TileContext(nc) as tc, ExitStack() as top:
        consts = top.enter_context(tc.tile_pool(name="consts", bufs=1))
        w1e_sb = consts.tile([Cin, EXT1], BF16)
        nc.sync.dma_start(w1e_sb[:], w1e[:, :])
        w2e_sb = consts.tile([128, C1 // 128, EXT2], BF16)
        nc.sync.dma_start(w2e_sb[:], w2e[:, :, :])
        id_sb = consts.tile([128, 128], F32)
        nc.sync.dma_start(id_sb[:], idf32[:, :])
        b1_sb = consts.tile([128, C1], F32)
        nc.sync.dma_start(b1_sb[:], bass.AP(tensor=b1.ap().tensor, offset=0,
                                            ap=[[0, 128], [1, C1]]))
        b2_sb = consts.tile([128, C2], F32)
        nc.sync.dma_start(b2_sb[:], bass.AP(tensor=b2.ap().tensor, offset=0,
                                            ap=[[0, 128], [1, C2]]))

        # ---------------- phase 1: layer-1 table (replicated linear)
        GRP = 8
        with ExitStack() as ph:
            xin = ph.enter_context(tc.tile_pool(name="xin2", bufs=2))
            pps = ph.enter_context(tc.tile_pool(name="pps2", bufs=4, space="PSUM"))
            dev = ph.enter_context(tc.tile_pool(name="dev", bufs=2))
            for g0 in range(0, n_tr, GRP):
                gts = list(range(g0, min(g0 + GRP, n_tr)))
                c0 = g0 * 128
                cn = min(slab - c0, GRP * 128)
                xg = xin.tile([Cin, GRP * 128], BF16, tag="xg2")
                nc.sync.dma_start(xg[:, :cn], xoT[:, c0:c0 + cn])
                db = dev.tile([128, GRP, 4], F32, tag="db")
                for k, t in enumerate(gts):
                    nn = min(128, slab - t * 128)
                    pp = pps.tile([128, 4], F32, tag="ppb")
                    nc.tensor.matmul(pp[:nn, :], lhsT=xg[:, k * 128:k * 128 + nn],
                                     rhs=w1e_sb[:, C1 + 4:C1 + 8],
                                     start=True, stop=True)
                    nc.vector.tensor_copy(db[:nn, k, :], pp[:nn, :])
                rows_write(nc.scalar, d1attn, c0, cn, db, 4, len(gts))

        with ExitStack() as ph:
            xin = ph.enter_context(tc.tile_pool(name="xin", bufs=3))
            pps = ph.enter_context(tc.tile_pool(name="pps", bufs=6, space="PSUM"))
            fev = ph.enter_context(tc.tile_pool(name="fev", bufs=3))
            for g0 in range(0, n_t1, GRP):
                gts = list(range(g0, min(g0 + GRP, n_t1)))
                c0 = g0 * 128
                cn = min(N - c0, GRP * 128)
                xg = xin.tile([Cin, GRP * 128], BF16, tag="xg")
                nc.sync.dma_start(xg[:, :cn], xT[:, c0:c0 + cn])
                fb = fev.tile([128, GRP, FW1], BF16, tag="fb")
                nc.vector.memset(fb[:, :, C1 + 8:], 0.0)
                for k, t in enumerate(gts):
                    nn = min(128, N - t * 128)
                    pp = pps.tile([128, EXT1], F32, tag="pp")
                    nc.tensor.matmul(pp[:nn, :], lhsT=xg[:, k * 128:k * 128 + nn],
                                     rhs=w1e_sb[:], start=True, stop=True)
                    if t % 2 == 0:
                        nc.vector.tensor_copy(fb[:nn, k, :C1], pp[:nn, :C1])
                    else:
                        nc.scalar.activation(fb[:nn, k, :C1], pp[:nn, :C1],
                                             mybir.ActivationFunctionType.Copy)
                    nc.vector.tensor_copy(
                        fb[:nn, k, C1:C1 + 8].bitcast(F32), pp[:nn, C1:C1 + 4])
                rows_write(nc.scalar, t1tab, c0, cn, fb, FW1, len(gts))
        def dummy_out():
            with ExitStack() as ph:
                zp = ph.enter_context(tc.tile_pool(name="zp", bufs=2))
                for r0 in range(0, slab, 128):
                    nn = min(128, slab - r0)
                    zt0 = zp.tile([128, C2], F32, tag="zt0")
                    nc.vector.memset(zt0[:], 0.0)
                    nc.scalar.dma_start(out[r0:r0 + nn, :], zt0[:nn, :])

        # ---------------- aggregation layers
        qcnt = [0]

        def agg_layer(st, layer, write_out=False, after_macro=None):
            C = C1 if layer == 1 else C2
            FW = FW1 if layer == 1 else FW2
            if layer == 1:
                IDX, SF, STF = IDX1, SF1, STF1
                tabA, tabB = t1tab[0:split, :], t1tab[split:, :]
            else:
                IDX, SF, STF = IDX2, SF2, STF2
                tabA, tabB = t2a[:, :], t2b[:, :]
            dtab = d1attn if layer == 1 else d2attn
            bb = b1_sb if layer == 1 else b2_sb
            with ExitStack() as ph:
                ixp = ph.enter_context(tc.tile_pool(name=f"ixp{layer}", bufs=3))
                ftp = ph.enter_context(tc.tile_pool(name=f"ftp{layer}", bufs=3))
                fp = ph.enter_context(tc.tile_pool(name=f"fp{layer}", bufs=2))
                sp = ph.enter_context(tc.tile_pool(name=f"sp{layer}", bufs=3))
                snp = ph.enter_context(tc.tile_pool(name=f"snp{layer}", bufs=3))
                np_ = ph.enter_context(tc.tile_pool(name=f"np{layer}", bufs=3))
                agg = ph.enter_context(
                    tc.tile_pool(name=f"agg{layer}", bufs=4 if layer == 1 else 6,
                                 space="PSUM"))
                bcp = ph.enter_context(
                    tc.tile_pool(name=f"bcp{layer}", bufs=2, space="PSUM"))
                if layer == 1:
                    tps = ph.enter_context(
                        tc.tile_pool(name=f"tps{layer}", bufs=1, space="PSUM"))
                    h2p = ph.enter_context(
                        tc.tile_pool(name=f"h2p{layer}", bufs=1, space="PSUM"))
                epi = ph.enter_context(tc.tile_pool(name=f"epi{layer}", bufs=4))
                l2e = ph.enter_context(tc.tile_pool(name=f"l2e{layer}", bufs=3))
                dsp = ph.enter_context(tc.tile_pool(name=f"dsp{layer}", bufs=1))

                # own-slab a_dst, node-major, resident for the whole layer
                dst_sb = dsp.tile([128, n_tr, 4], F32)
                rows_read(nc.sync, dtab, 0, slab, dst_sb, 4)
                dstb = dsp.tile([128, n_tr, 4], BF16)
                nc.vector.tensor_copy(dstb[:], dst_sb[:])

                for mi, mm in enumerate(st.macros):
                    nblk = mm.nblk
                    cA = mm.ci_B - mm.ci_A
                    cB = mm.ci_D - mm.ci_B
                    wtot = cA + cB
                    ix = ixp.tile([128, wtot], I16, tag="ix")
                    nc.sync.dma_start(ix[:], IDX[:, mm.ci_A:mm.ci_A + wtot])
                    S = sp.tile([128, nblk, 128], FP8, tag="S")
                    nc.sync.dma_start(
                        S[:], SF[:, mm.co * 128:(mm.co + nblk) * 128])
                    ST = snp.tile([128, nblk, 128], FP8, tag="ST")
                    nc.scalar.dma_start(
                        ST[:], STF[:, mm.co * 128:(mm.co + nblk) * 128])
                    ft = ftp.tile([128, nblk, FW], BF16, tag="ft")

                    # a_dst broadcast to edges via per-block 4-col matmuls
                    # (lhsT = host-provided one-hot S^T); independent of the
                    # gathers, so emitted first
                    ebp = bcp.tile([128, nblk, 4], F32, tag="ebp")
                    for j, (ri, _, _) in enumerate(mm.blocks):
                        nc.tensor.matmul(
                            ebp[:, j, :], lhsT=ST[:, j, :],
                            rhs=dstb[:, mm.ranges[ri], :],
                            start=True, stop=True)

                    banks = [agg.tile([128, C + 4], F32, tag="bank",
                                      name=f"bank{ri}")
                             for ri in range(len(mm.ranges))]
                    et = np_.tile([128, nblk, 4], F32, tag="et")
                    e2 = np_.tile([128, nblk, 4], F32, tag="e2")
                    net = np_.tile([128, nblk, 4], BF16, tag="net")
                    mz = fp.tile([128, nblk, C + 4], BF16, tag="mz")
                    CH = C // HEADS
                    av = ft[:, :, C:C + 8].bitcast(F32)   # [128, nblk, 4] a_src

                    def att(b0, qn):
                        """attention for blocks [b0, b0+qn): leaky-relu of
                        a_src + a_dst, then exp (on ACT)"""
                        sl = slice(b0, b0 + qn)
                        nc.vector.tensor_tensor(et[:, sl, :], av[:, sl, :],
                                                ebp[:, sl, :],
                                                op=mybir.AluOpType.add)
                        nc.vector.scalar_tensor_tensor(
                            et[:, sl, :], et[:, sl, :], NEG, et[:, sl, :],
                            op0=mybir.AluOpType.mult, op1=mybir.AluOpType.max)
                        nc.scalar.activation(net[:, sl, :], et[:, sl, :],
                                             mybir.ActivationFunctionType.Exp)
                        nc.scalar.activation(mz[:, sl, C:C + 4], net[:, sl, :],
                                             mybir.ActivationFunctionType.Copy)

                    def grp(b0, qn):
                        """messages + aggregation for blocks [b0, b0+qn)"""
                        sl = slice(b0, b0 + qn)
                        for h in range(HEADS):
                            nc.vector.tensor_tensor(
                                mz[:, sl, h * CH:(h + 1) * CH],
                                ft[:, sl, h * CH:(h + 1) * CH],
                                net[:, sl, h:h + 1].broadcast_to(
                                    [128, qn, CH]),
                                op=mybir.AluOpType.mult)
                        for j in range(b0, b0 + qn):
                            ri, first, last = mm.blocks[j]
                            nc.tensor.matmul(banks[ri][:], lhsT=S[:, j, :],
                                             rhs=mz[:, j, :],
                                             start=first, stop=last)

                    def gat(b0, nb, tab, icol0):
                        ncalls = min(4, nb)
                        sz = -(-nb // ncalls)
                        for q0 in range(0, nb, sz):
                            qn = min(sz, nb - q0)
                            nc.gpsimd.dma_gather(
                                out_ap=ft[:, b0 + q0:b0 + q0 + qn, :],
                                in_ap=tab,
                                idxs_ap=ix[:, icol0 + q0 * 8:
                                           icol0 + (q0 + qn) * 8],
                                num_idxs=qn * 128,
                                num_idxs_reg=nreg(qn * 128), elem_size=FW,
                                queue_num=qcnt[0] % 4)
                            qcnt[0] += 1
                            nc.tensor.ldweights(ft[:, b0 + q0, 0:128])
                        att(b0, nb)
                        ncalls = min(4, nb)
                        sz = -(-nb // ncalls)
                        for q0 in range(0, nb, sz):
                            qn = min(sz, nb - q0)
                            grp(b0 + q0, qn)

                    if mm.nA:
                        gat(0, mm.nA, tabA, 0)
                    if mm.nB:
                        gat(mm.nA, mm.nB, tabB, cA)
                    if mm.selfs:
                        # layer-2 self-loop rows come straight from the local
                        # pre-AllGather table -- no gather needed
                        for js, ri in mm.selfs:
                            w = min(mm.node_base[ri], slab - 128)
                            nc.sync.dma_start(
                                ft[:, js, :],
                                bass.AP(tensor=agf_in.ap().tensor,
                                        offset=w * FW,
                                        ap=[[FW, 128], [1, FW]]))
                        att(mm.nA + mm.nB, len(mm.selfs))
                        grp(mm.nA + mm.nB, len(mm.selfs))

                    for ri in range(len(mm.ranges)):
                        nn = mm.n_nodes[ri]
                        base = mm.node_base[ri]
                        bank = banks[ri]
                        zt = epi.tile([128, 4], F32, tag="zt")
                        nc.vector.tensor_scalar_add(zt[:], bank[:, C:C + 4], 1e-30)
                        zi = epi.tile([128, 4], F32, tag="zi")
                        nc.vector.reciprocal(zi[:], zt[:])
                        avg = epi.tile([128, C], F32, tag="avg")
                        nc.vector.tensor_tensor(
                            avg[:].rearrange("p (h c) -> p h c", h=HEADS),
                            bank[:, 0:C].rearrange("p (h c) -> p h c", h=HEADS),
                            zi[:].broadcast_to([128, HEADS, C // HEADS]),
                            op=mybir.AluOpType.mult)
                        nc.vector.tensor_tensor(avg[:], avg[:], bb[:],
                                                op=mybir.AluOpType.add)
                        if write_out:
                            nc.scalar.dma_start(out[base:base + nn, :],
                                                avg[:nn, :])
                        if layer == 2:
                            nc.scalar.dma_start(out[base:base + nn, :],
                                                avg[:nn, :])
                            continue
                        if SKIP_L2PREP:
                            continue
                        r = epi.tile([128, C], F32, tag="relu")
                        nc.scalar.activation(r[:], avg[:],
                                             mybir.ActivationFunctionType.Relu)
                        rT = l2e.tile([128, C // 128, 128], BF16, tag="rT")
                        tp = tps.tile([128, C // 128, 128], F32, tag="tp1")
                        for c in range(C // 128):
                            nc.tensor.transpose(tp[:, c, :],
                                                r[:, c * 128:(c + 1) * 128],
                                                id_sb[:])
                        nc.vector.tensor_copy(rT[:], tp[:])
                        h2 = h2p.tile([128, EXT2], F32, tag="h2")
                        for c in range(C // 128):
                            nc.tensor.matmul(h2[:], lhsT=rT[:, c, :],
                                             rhs=w2e_sb[:, c, :],
                                             start=(c == 0),
                                             stop=(c == C // 128 - 1))
                        fe = l2e.tile([128, FW2], BF16, tag="fe")
                        nc.vector.memset(fe[:, C2 + 8:], 0.0)
                        nc.scalar.activation(fe[:, :C2], h2[:, 0:C2],
                                             mybir.ActivationFunctionType.Copy)
                        nc.vector.tensor_copy(fe[:, C2:C2 + 8].bitcast(F32),
                                              h2[:, C2:C2 + 4])
                        nc.scalar.dma_start(agf_in[base:base + nn, :], fe[:nn, :])
                        sa = l2e.tile([128, 4], F32, tag="sa")
                        nc.vector.tensor_copy(sa[:], h2[:, C2 + 4:C2 + 8])
                        nc.sync.dma_start(d2attn[base:base + nn, :], sa[:nn, :])
                    if after_macro is not None:
                        after_macro(mi)

        def ag_cb(mi):
            if mi == 12:
                nc.gpsimd.collective_compute(
                    "AllGather", mybir.AluOpType.bypass,
                    ins=[agf_in[0:CH0, :]], outs=[t2a[:]],
                    replica_groups=[list(range(NCORES))])

        if stage == 1:
            dummy_out()
        if stage >= 2:
            agg_layer(st1, 1, write_out=(stage == 2),
                      after_macro=ag_cb if stage >= 3 else None)
        if stage >= 3:
            nc.gpsimd.collective_compute(
                "AllGather", mybir.AluOpType.bypass,
                ins=[agf_in[CH0:slab, :]], outs=[t2b[:]],
                replica_groups=[list(range(NCORES))])
        if stage == 3:
            dummy_out()
        if stage >= 4:
            agg_layer(st2, 2)

    nc.compile()
    return nc


# ---------------------------------------------------------------- entry point

def _run(x, edge_index, W1, att_src1, att_dst1, b1, W2, att_src2, att_dst2, b2,
         split=32768):
    x = np.asarray(x, np.float32)
    N, Cin = x.shape
    C1 = W1.shape[1]
    C2 = W2.shape[1]
    slab = N // NCORES

    loops = np.arange(N, dtype=np.int64)
    src = np.concatenate([np.asarray(edge_index[0], np.int64), loops])
    dst = np.concatenate([np.asarray(edge_index[1], np.int64), loops])

    CH0 = 3328
    CH1 = slab - CH0

    def grp1(s):
        return s >= split

    def idx1(s):
        return np.where(s >= split, s - split, s)

    def grp2(s):
        return (s % slab) >= CH0

    def idx2(s):
        r = s // slab
        l = s % slab
        return np.where(l >= CH0, r * CH1 + (l - CH0), r * CH0 + l)

    st1 = _build_structure(src, dst, N, slab, grp1, idx1)
    st2 = _build_structure(src, dst, N, slab, grp2, idx2, self_direct=True)
    nc = _build_nc(st1, st2, N, slab, C1, C2, Cin)

    W1e = _fold_ext(np.asarray(W1, np.float32), np.asarray(att_src1, np.float32),
                    np.asarray(att_dst1, np.float32)).astype(bf16)
    W2e = _fold_ext(np.asarray(W2, np.float32), np.asarray(att_src2, np.float32),
                    np.asarray(att_dst2, np.float32)).astype(bf16)
    w2e_arr = np.ascontiguousarray(
        W2e.reshape(C1 // 128, 128, C2 + 2 * HEADS).transpose(1, 0, 2))
    xTb = np.ascontiguousarray(x.T.astype(bf16))
    ident = np.eye(128, dtype=np.float32)

    in_maps = []
    for core in range(NCORES):
        sl = slice(core * slab, (core + 1) * slab)
        in_maps.append({
            "xT": xTb,
            "xoT": np.ascontiguousarray(xTb[:, sl]),
            "w1e": W1e,
            "w2e": w2e_arr,
            "b1": np.asarray(b1, np.float32).reshape(1, C1),
            "b2": np.asarray(b2, np.float32).reshape(1, C2),
            "idf32": ident,
            "IDX1": st1.IDXs[core],
            "SF1": st1.SFs[core],
            "STF1": st1.STFs[core],
            "IDX2": st2.IDXs[core],
            "SF2": st2.SFs[core],
            "STF2": st2.STFs[core],
        })

    res = run_bass_kernel_spmd(nc, in_maps, core_ids=list(range(NCORES)),
                               trace=TRACE)
    if TRACE:
        global LAST_RESULT
        LAST_RESULT = res
    outv = np.concatenate([res.results[c]["out"] for c in range(NCORES)], axis=0)
    return outv.astype(np.float32)


def kernel(x, edge_index, W1, att_src1, att_dst1, b1, W2, att_src2, att_dst2,
           b2):
    return _run(x, edge_index, W1, att_src1, att_dst1, b1,
                W2, att_src2, att_dst2, b2)



# revision 57
# speedup vs baseline: 1.5764x; 1.5764x over previous
"""Two-layer GAT on 8 Trainium2 NeuronCores (Bass/Tile).

Strategy (graph/data parallel, per sharding hint):
  - Nodes sharded by destination across 8 cores (slab = N/8 each).
  - Per layer, per edge we gather from HBM tables:
      feat-table  [N, 256] bf16  (pre-transformed features x@W1 / relu(h1)@W2)
      attn-table  [N, 64] f32    (a_src in cols 0:4; 256B rows for dma_gather)
      dst-table   [SLAB, 64] f32 (a_dst of own nodes, local dst index)
  - Edges grouped into 128-aligned destination node ranges (SPMD-uniform
    structure), each range's edges split into A (src < 32768) / B groups for
    int16 gather indices, padded to 128-slot sub-blocks.
  - Per 128-edge sub-block: host-provided static fp8 one-hot S / S^T
    matrices (DMA-streamed); TensorE matmuls broadcast a_dst to edges (S^T)
    and reduce softmax-weighted messages + z into PSUM node-major
    accumulators. No segment max (exp-safe value range). Gathers ride 4
    SWDGE queues; attention/messages/aggregation are streamed per gather
    group behind the gathers.
  - Layer 1 epilogue computes relu(h1) and immediately the layer-2 table row
    chunk (transpose + W2_ext matmul); slabs are AllGather'd between layers.
"""

import numpy as np
import ml_dtypes
from contextlib import ExitStack

import concourse.bass as bass
import concourse.tile as tile
import concourse.bacc as bacc
from concourse import mybir
from concourse.bass_utils import run_bass_kernel_spmd

bf16 = ml_dtypes.bfloat16
F32 = mybir.dt.float32
BF16 = mybir.dt.bfloat16
I16 = mybir.dt.int16
FP8 = mybir.dt.float8e4
fp8 = ml_dtypes.float8_e4m3

NCORES = 8
HEADS = 4
TRACE = False
LAST_RESULT = None
NEG = 0.2
RANGES_PER_MACRO = 2
SKIP_L2PREP = False
AGG_CUT = 'full'


# ---------------------------------------------------------------- host prep

def _fold_ext(W, att_src, att_dst):
    H, C = att_src.shape
    Ws = (W.reshape(W.shape[0], H, C) * att_src[None]).sum(-1)
    Wd = (W.reshape(W.shape[0], H, C) * att_dst[None]).sum(-1)
    return np.concatenate([W, Ws, Wd], axis=1)  # [Cin, Cout + 2H]


def _wrap16(idx):
    """int16 idx list (len % 16 == 0) -> [128, len/16] wrapped+replicated."""
    w = np.asarray(idx, np.int16).reshape(-1, 16).T  # [16, len/16]
    return np.tile(w, (8, 1))


class Meta:
    pass


def _build_structure(src, dst, N, slab, grp_of, idx_of):
    """Uniform (cross-core) macro/range/sub-block structure + per-core arrays.

    grp_of(s) -> bool array (False=table A, True=table B);
    idx_of(s) -> int16-range row index within that table."""
    n_ranges = -(-slab // 128)
    per_core = []
    for core in range(NCORES):
        lo = core * slab
        m = (dst >= lo) & (dst < lo + slab)
        s = src[m].astype(np.int64)
        d = (dst[m] - lo).astype(np.int64)
        order = np.argsort(d, kind="stable")
        s, d = s[order], d[order]
        starts = np.concatenate([[0], np.cumsum(np.bincount(d, minlength=slab))])
        per_core.append((s, d, starts))

    # per-range A/B slot budgets: max over cores, rounded up to 128
    A_r = np.zeros(n_ranges, np.int64)
    B_r = np.zeros(n_ranges, np.int64)
    for core in range(NCORES):
        s, d, starts = per_core[core]
        for r in range(n_ranges):
            e0, e1 = starts[r * 128], starts[min((r + 1) * 128, slab)]
            es = s[e0:e1]
            a = int((~grp_of(es)).sum())
            A_r[r] = max(A_r[r], a)
            B_r[r] = max(B_r[r], e1 - e0 - a)
    A_r = (-(-A_r // 128) * 128).astype(np.int64)
    B_r = (-(-B_r // 128) * 128).astype(np.int64)

    # macros: groups of ranges
    macros = []
    for m0 in range(0, n_ranges, RANGES_PER_MACRO):
        rs = list(range(m0, min(m0 + RANGES_PER_MACRO, n_ranges)))
        mm = Meta()
        mm.ranges = rs
        mm.nA = int(sum(A_r[r] for r in rs)) // 128   # A sub-blocks
        mm.nB = int(sum(B_r[r] for r in rs)) // 128
        mm.nblk = mm.nA + mm.nB
        blocks = []
        for i, r in enumerate(rs):
            for _ in range(A_r[r] // 128):
                blocks.append([i, False, False])
        for i, r in enumerate(rs):
            for _ in range(B_r[r] // 128):
                blocks.append([i, False, False])
        seen = set()
        for b in blocks:
            if b[0] not in seen:
                b[1] = True
                seen.add(b[0])
        last = {}
        for k, b in enumerate(blocks):
            last[b[0]] = k
        for i, k in last.items():
            blocks[k][2] = True
        mm.blocks = blocks
        mm.n_nodes = [min(128, slab - r * 128) for r in rs]
        mm.node_base = [r * 128 for r in rs]
        macros.append(mm)

    col_i = 0
    col_o = 0
    for mm in macros:
        mm.ci_A = col_i
        col_i += (mm.nA * 128) // 16
        mm.ci_B = col_i
        col_i += (mm.nB * 128) // 16
        mm.ci_D = col_i
        col_i += (mm.nblk * 128) // 16
        mm.co = col_o
        col_o += mm.nblk
    WI, WO = col_i, col_o

    IDXs, SFs, STFs = [], [], []
    rng128 = np.arange(128)
    for core in range(NCORES):
        s, d, starts = per_core[core]
        IDX = np.zeros((128, WI), np.int16)
        SFa = np.zeros((128, WO * 128), fp8)
        STFa = np.zeros((128, WO * 128), fp8)
        for mm in macros:
            idxA, idxB, oA, oB, dA, dB = [], [], [], [], [], []
            for i, r in enumerate(mm.ranges):
                e0, e1 = starts[r * 128], starts[min((r + 1) * 128, slab)]
                es, ed = s[e0:e1], d[e0:e1]
                selA = ~grp_of(es)
                padA = int(A_r[r] - selA.sum())
                padB = int(B_r[r] - (~selA).sum())
                idxA += list(idx_of(es[selA])) + [0] * padA
                oA += list(ed[selA] - r * 128) + [-1] * padA
                dA += list(ed[selA]) + [0] * padA
                idxB += list(idx_of(es[~selA])) + [0] * padB
                oB += list(ed[~selA] - r * 128) + [-1] * padB
                dB += list(ed[~selA]) + [0] * padB
            if mm.nA:
                IDX[:, mm.ci_A:mm.ci_B] = _wrap16(idxA)
            if mm.nB:
                IDX[:, mm.ci_B:mm.ci_D] = _wrap16(idxB)
            IDX[:, mm.ci_D:mm.ci_D + (mm.nblk * 128) // 16] = _wrap16(dA + dB)
            oo = np.array(oA + oB, np.int32).reshape(mm.nblk, 128)
            oh = (oo[:, :, None] == rng128[None, None, :])  # [j, e, n]
            SFa[:, mm.co * 128:(mm.co + mm.nblk) * 128] = \
                oh.transpose(1, 0, 2).reshape(128, -1).astype(fp8)
            STFa[:, mm.co * 128:(mm.co + mm.nblk) * 128] = \
                oh.transpose(2, 0, 1).reshape(128, -1).astype(fp8)
        IDXs.append(IDX)
        SFs.append(SFa)
        STFs.append(STFa)

    st = Meta()
    st.n_ranges = n_ranges
    st.macros = macros
    st.WI, st.WO = WI, WO
    st.IDXs, st.SFs, st.STFs = IDXs, SFs, STFs
    return st


# ---------------------------------------------------------------- kernel build

def _mid_bcast(ap, count, pos):
    """Insert a [0, count] dim at position `pos` of an AP's ap list."""
    a = ap.ap.copy()
    a.insert(pos, [0, count])
    return bass.AP(tensor=ap.tensor, offset=ap.offset, ap=a)


def _build_nc(st1, st2, N, slab, C1, C2, Cin, stage=4):
    nc = bacc.Bacc("TRN2", target_bir_lowering=False, debug=False,
                   num_devices=NCORES, num_swdge_queues=4)
    EXT1 = C1 + 2 * HEADS
    EXT2 = C2 + 2 * HEADS
    FW1 = C1 + 128          # folded feat+attn table row (bf16 cols, 256B mult)
    FW2 = C2 + 128
    n_t1 = -(-N // 128)
    n_tr = st1.n_ranges
    split = 32768
    CH0 = 3328               # layer-2 AllGather chunk boundary (local rows)
    CH1 = slab - CH0

    xT = nc.dram_tensor("xT", [Cin, N], BF16, kind="ExternalInput")
    xoT = nc.dram_tensor("xoT", [Cin, slab], BF16, kind="ExternalInput")
    w1e = nc.dram_tensor("w1e", [Cin, EXT1], BF16, kind="ExternalInput")
    w2e = nc.dram_tensor("w2e", [128, C1 // 128, EXT2], BF16, kind="ExternalInput")
    b1 = nc.dram_tensor("b1", [1, C1], F32, kind="ExternalInput")
    b2 = nc.dram_tensor("b2", [1, C2], F32, kind="ExternalInput")
    idf32 = nc.dram_tensor("idf32", [128, 128], F32, kind="ExternalInput")
    IDX1 = nc.dram_tensor("IDX1", [128, st1.WI], I16, kind="ExternalInput")
    SF1 = nc.dram_tensor("SF1", [128, st1.WO * 128], FP8, kind="ExternalInput")
    STF1 = nc.dram_tensor("STF1", [128, st1.WO * 128], FP8,
                          kind="ExternalInput")
    IDX2 = nc.dram_tensor("IDX2", [128, st2.WI], I16, kind="ExternalInput")
    SF2 = nc.dram_tensor("SF2", [128, st2.WO * 128], FP8, kind="ExternalInput")
    STF2 = nc.dram_tensor("STF2", [128, st2.WO * 128], FP8,
                          kind="ExternalInput")

    t1tab = nc.dram_tensor("t1tab", [N, FW1], BF16)
    d1attn = nc.dram_tensor("d1attn", [slab, 4], F32)
    agf_in = nc.dram_tensor("agf_in", [slab, FW2], BF16)
    t2a = nc.dram_tensor("t2a", [NCORES * CH0, FW2], BF16, addr_space="Shared")
    t2b = nc.dram_tensor("t2b", [NCORES * CH1, FW2], BF16, addr_space="Shared")
    d2attn = nc.dram_tensor("d2attn", [slab, 4], F32)
    out = nc.dram_tensor("out", [slab, C2], F32, kind="ExternalOutput")

    reg_cache = {}

    def nreg(v):
        if v not in reg_cache:
            reg_cache[v] = nc.gpsimd.to_reg(v)
        return reg_cache[v]

    def rows_write(eng, dram, row0, total_rows, sb_tile, width, ntiles):
        full = total_rows // 128
        rem = total_rows - full * 128
        if full:
            eng.dma_start(
                bass.AP(tensor=dram.ap().tensor, offset=row0 * width,
                        ap=[[width, 128], [128 * width, full], [1, width]]),
                sb_tile[:, 0:full, :])
        if rem:
            eng.dma_start(
                bass.AP(tensor=dram.ap().tensor,
                        offset=(row0 + full * 128) * width,
                        ap=[[width, rem], [1, width]]),
                sb_tile[:rem, full, :])

    def rows_read(eng, dram, row0, total_rows, sb_tile, width):
        full = total_rows // 128
        rem = total_rows - full * 128
        if full:
            eng.dma_start(
                sb_tile[:, 0:full, :],
                bass.AP(tensor=dram.ap().tensor, offset=row0 * width,
                        ap=[[width, 128], [128 * width, full], [1, width]]))
        if rem:
            eng.dma_start(
                sb_tile[:rem, full, :],
                bass.AP(tensor=dram.ap().tensor,
                        offset=(row0 + full * 128) * width,
                        ap=[[width, rem], [1, width]]))

    with tile.TileContext(nc) as tc, ExitStack() as top:
        consts = top.enter_context(tc.tile_pool(name="consts", bufs=1))
        w1e_sb = consts.tile([Cin, EXT1], BF16)
        nc.sync.dma_start(w1e_sb[:], w1e[:, :])
        w2e_sb = consts.tile([128, C1 // 128, EXT2], BF16)
        nc.sync.dma_start(w2e_sb[:], w2e[:, :, :])
        id_sb = consts.tile([128, 128], F32)
        nc.sync.dma_start(id_sb[:], idf32[:, :])
        b1_sb = consts.tile([128, C1], F32)
        nc.sync.dma_start(b1_sb[:], bass.AP(tensor=b1.ap().tensor, offset=0,
                                            ap=[[0, 128], [1, C1]]))
        b2_sb = consts.tile([128, C2], F32)
        nc.sync.dma_start(b2_sb[:], bass.AP(tensor=b2.ap().tensor, offset=0,
                                            ap=[[0, 128], [1, C2]]))

        # ---------------- phase 1: layer-1 table (replicated linear)
        GRP = 8
        with ExitStack() as ph:
            xin = ph.enter_context(tc.tile_pool(name="xin", bufs=2))
            pps = ph.enter_context(tc.tile_pool(name="pps", bufs=4, space="PSUM"))
            fev = ph.enter_context(tc.tile_pool(name="fev", bufs=2))
            for g0 in range(0, n_t1, GRP):
                gts = list(range(g0, min(g0 + GRP, n_t1)))
                c0 = g0 * 128
                cn = min(N - c0, GRP * 128)
                xg = xin.tile([Cin, GRP * 128], BF16, tag="xg")
                nc.sync.dma_start(xg[:, :cn], xT[:, c0:c0 + cn])
                fb = fev.tile([128, GRP, FW1], BF16, tag="fb")
                nc.vector.memset(fb[:, :, C1 + 8:], 0.0)
                for k, t in enumerate(gts):
                    nn = min(128, N - t * 128)
                    pp = pps.tile([128, EXT1], F32, tag="pp")
                    nc.tensor.matmul(pp[:nn, :], lhsT=xg[:, k * 128:k * 128 + nn],
                                     rhs=w1e_sb[:], start=True, stop=True)
                    if t % 2 == 0:
                        nc.vector.tensor_copy(fb[:nn, k, :C1], pp[:nn, :C1])
                    else:
                        nc.scalar.activation(fb[:nn, k, :C1], pp[:nn, :C1],
                                             mybir.ActivationFunctionType.Copy)
                    nc.vector.tensor_copy(
                        fb[:nn, k, C1:C1 + 8].bitcast(F32), pp[:nn, C1:C1 + 4])
                rows_write(nc.scalar, t1tab, c0, cn, fb, FW1, len(gts))
        with ExitStack() as ph:
            xin = ph.enter_context(tc.tile_pool(name="xin2", bufs=2))
            pps = ph.enter_context(tc.tile_pool(name="pps2", bufs=4, space="PSUM"))
            dev = ph.enter_context(tc.tile_pool(name="dev", bufs=2))
            for g0 in range(0, n_tr, GRP):
                gts = list(range(g0, min(g0 + GRP, n_tr)))
                c0 = g0 * 128
                cn = min(slab - c0, GRP * 128)
                xg = xin.tile([Cin, GRP * 128], BF16, tag="xg2")
                nc.sync.dma_start(xg[:, :cn], xoT[:, c0:c0 + cn])
                db = dev.tile([128, GRP, 4], F32, tag="db")
                for k, t in enumerate(gts):
                    nn = min(128, slab - t * 128)
                    pp = pps.tile([128, 4], F32, tag="ppb")
                    nc.tensor.matmul(pp[:nn, :], lhsT=xg[:, k * 128:k * 128 + nn],
                                     rhs=w1e_sb[:, C1 + 4:C1 + 8],
                                     start=True, stop=True)
                    nc.vector.tensor_copy(db[:nn, k, :], pp[:nn, :])
                rows_write(nc.scalar, d1attn, c0, cn, db, 4, len(gts))

        def dummy_out():
            with ExitStack() as ph:
                zp = ph.enter_context(tc.tile_pool(name="zp", bufs=2))
                for r0 in range(0, slab, 128):
                    nn = min(128, slab - r0)
                    zt0 = zp.tile([128, C2], F32, tag="zt0")
                    nc.vector.memset(zt0[:], 0.0)
                    nc.scalar.dma_start(out[r0:r0 + nn, :], zt0[:nn, :])

        # ---------------- aggregation layers
        qcnt = [0]

        def agg_layer(st, layer, write_out=False, after_macro=None):
            C = C1 if layer == 1 else C2
            FW = FW1 if layer == 1 else FW2
            if layer == 1:
                IDX, SF, STF = IDX1, SF1, STF1
                tabA, tabB = t1tab[0:split, :], t1tab[split:, :]
            else:
                IDX, SF, STF = IDX2, SF2, STF2
                tabA, tabB = t2a[:, :], t2b[:, :]
            dtab = d1attn if layer == 1 else d2attn
            bb = b1_sb if layer == 1 else b2_sb
            with ExitStack() as ph:
                ixp = ph.enter_context(tc.tile_pool(name=f"ixp{layer}", bufs=3))
                ftp = ph.enter_context(tc.tile_pool(name=f"ftp{layer}", bufs=3))
                fp = ph.enter_context(tc.tile_pool(name=f"fp{layer}", bufs=2))
                sp = ph.enter_context(tc.tile_pool(name=f"sp{layer}", bufs=3))
                snp = ph.enter_context(tc.tile_pool(name=f"snp{layer}", bufs=3))
                np_ = ph.enter_context(tc.tile_pool(name=f"np{layer}", bufs=3))
                agg = ph.enter_context(
                    tc.tile_pool(name=f"agg{layer}", bufs=3 if layer == 1 else 6,
                                 space="PSUM"))
                bcp = ph.enter_context(
                    tc.tile_pool(name=f"bcp{layer}", bufs=2, space="PSUM"))
                if layer == 1:
                    tps = ph.enter_context(
                        tc.tile_pool(name=f"tps{layer}", bufs=1, space="PSUM"))
                    h2p = ph.enter_context(
                        tc.tile_pool(name=f"h2p{layer}", bufs=2, space="PSUM"))
                epi = ph.enter_context(tc.tile_pool(name=f"epi{layer}", bufs=3))
                l2e = ph.enter_context(tc.tile_pool(name=f"l2e{layer}", bufs=2))
                dsp = ph.enter_context(tc.tile_pool(name=f"dsp{layer}", bufs=1))

                # own-slab a_dst, node-major, resident for the whole layer
                dst_sb = dsp.tile([128, n_tr, 4], F32)
                rows_read(nc.sync, dtab, 0, slab, dst_sb, 4)
                dstb = dsp.tile([128, n_tr, 4], BF16)
                nc.vector.tensor_copy(dstb[:], dst_sb[:])

                for mi, mm in enumerate(st.macros):
                    nblk = mm.nblk
                    cA = mm.ci_B - mm.ci_A
                    cB = mm.ci_D - mm.ci_B
                    wtot = cA + cB
                    ix = ixp.tile([128, wtot], I16, tag="ix")
                    nc.sync.dma_start(ix[:], IDX[:, mm.ci_A:mm.ci_A + wtot])
                    S = sp.tile([128, nblk, 128], FP8, tag="S")
                    nc.sync.dma_start(
                        S[:], SF[:, mm.co * 128:(mm.co + nblk) * 128])
                    ST = snp.tile([128, nblk, 128], FP8, tag="ST")
                    nc.scalar.dma_start(
                        ST[:], STF[:, mm.co * 128:(mm.co + nblk) * 128])
                    ft = ftp.tile([128, nblk, FW], BF16, tag="ft")

                    # a_dst broadcast to edges via per-block 4-col matmuls
                    # (lhsT = host-provided one-hot S^T); independent of the
                    # gathers, so emitted first
                    ebp = bcp.tile([128, nblk, 4], F32, tag="ebp")
                    for j, (ri, _, _) in enumerate(mm.blocks):
                        nc.tensor.matmul(
                            ebp[:, j, :], lhsT=ST[:, j, :],
                            rhs=dstb[:, mm.ranges[ri], :],
                            start=True, stop=True)

                    banks = [agg.tile([128, C + 4], F32, tag="bank",
                                      name=f"bank{ri}")
                             for ri in range(len(mm.ranges))]
                    et = np_.tile([128, nblk, 4], F32, tag="et")
                    e2 = np_.tile([128, nblk, 4], F32, tag="e2")
                    net = np_.tile([128, nblk, 4], BF16, tag="net")
                    mz = fp.tile([128, nblk, C + 4], BF16, tag="mz")
                    CH = C // HEADS
                    av = ft[:, :, C:C + 8].bitcast(F32)   # [128, nblk, 4] a_src

                    def att(b0, qn):
                        """attention for blocks [b0, b0+qn): leaky-relu of
                        a_src + a_dst, then exp (on ACT)"""
                        sl = slice(b0, b0 + qn)
                        nc.vector.tensor_tensor(et[:, sl, :], av[:, sl, :],
                                                ebp[:, sl, :],
                                                op=mybir.AluOpType.add)
                        nc.vector.scalar_tensor_tensor(
                            et[:, sl, :], et[:, sl, :], NEG, et[:, sl, :],
                            op0=mybir.AluOpType.mult, op1=mybir.AluOpType.max)
                        nc.scalar.activation(net[:, sl, :], et[:, sl, :],
                                             mybir.ActivationFunctionType.Exp)
                        nc.scalar.activation(mz[:, sl, C:C + 4], net[:, sl, :],
                                             mybir.ActivationFunctionType.Copy)

                    def grp(b0, qn):
                        """messages + aggregation for blocks [b0, b0+qn)"""
                        sl = slice(b0, b0 + qn)
                        for h in range(HEADS):
                            nc.vector.tensor_tensor(
                                mz[:, sl, h * CH:(h + 1) * CH],
                                ft[:, sl, h * CH:(h + 1) * CH],
                                net[:, sl, h:h + 1].broadcast_to(
                                    [128, qn, CH]),
                                op=mybir.AluOpType.mult)
                        for j in range(b0, b0 + qn):
                            ri, first, last = mm.blocks[j]
                            nc.tensor.matmul(banks[ri][:], lhsT=S[:, j, :],
                                             rhs=mz[:, j, :],
                                             start=first, stop=last)

                    def gat(b0, nb, tab, icol0):
                        ncalls = min(4, nb)
                        sz = -(-nb // ncalls)
                        for q0 in range(0, nb, sz):
                            qn = min(sz, nb - q0)
                            nc.gpsimd.dma_gather(
                                out_ap=ft[:, b0 + q0:b0 + q0 + qn, :],
                                in_ap=tab,
                                idxs_ap=ix[:, icol0 + q0 * 8:
                                           icol0 + (q0 + qn) * 8],
                                num_idxs=qn * 128,
                                num_idxs_reg=nreg(qn * 128), elem_size=FW,
                                queue_num=qcnt[0] % 4)
                            qcnt[0] += 1
                            nc.tensor.ldweights(ft[:, b0 + q0, 0:128])
                        att(b0, nb)
                        ncalls = min(4, nb)
                        sz = -(-nb // ncalls)
                        for q0 in range(0, nb, sz):
                            qn = min(sz, nb - q0)
                            grp(b0 + q0, qn)

                    if mm.nA:
                        gat(0, mm.nA, tabA, 0)
                    if mm.nB:
                        gat(mm.nA, mm.nB, tabB, cA)

                    for ri in range(len(mm.ranges)):
                        nn = mm.n_nodes[ri]
                        base = mm.node_base[ri]
                        bank = banks[ri]
                        zt = epi.tile([128, 4], F32, tag="zt")
                        nc.vector.tensor_scalar_add(zt[:], bank[:, C:C + 4], 1e-30)
                        zi = epi.tile([128, 4], F32, tag="zi")
                        nc.vector.reciprocal(zi[:], zt[:])
                        avg = epi.tile([128, C], F32, tag="avg")
                        nc.vector.tensor_tensor(
                            avg[:].rearrange("p (h c) -> p h c", h=HEADS),
                            bank[:, 0:C].rearrange("p (h c) -> p h c", h=HEADS),
                            zi[:].broadcast_to([128, HEADS, C // HEADS]),
                            op=mybir.AluOpType.mult)
                        nc.vector.tensor_tensor(avg[:], avg[:], bb[:],
                                                op=mybir.AluOpType.add)
                        if write_out:
                            nc.scalar.dma_start(out[base:base + nn, :],
                                                avg[:nn, :])
                        if layer == 2:
                            nc.scalar.dma_start(out[base:base + nn, :],
                                                avg[:nn, :])
                            continue
                        if SKIP_L2PREP:
                            continue
                        r = epi.tile([128, C], F32, tag="relu")
                        nc.scalar.activation(r[:], avg[:],
                                             mybir.ActivationFunctionType.Relu)
                        rT = l2e.tile([128, C // 128, 128], BF16, tag="rT")
                        tp = tps.tile([128, C // 128, 128], F32, tag="tp1")
                        for c in range(C // 128):
                            nc.tensor.transpose(tp[:, c, :],
                                                r[:, c * 128:(c + 1) * 128],
                                                id_sb[:])
                        nc.vector.tensor_copy(rT[:], tp[:])
                        h2 = h2p.tile([128, EXT2], F32, tag="h2")
                        for c in range(C // 128):
                            nc.tensor.matmul(h2[:], lhsT=rT[:, c, :],
                                             rhs=w2e_sb[:, c, :],
                                             start=(c == 0),
                                             stop=(c == C // 128 - 1))
                        fe = l2e.tile([128, FW2], BF16, tag="fe")
                        nc.vector.memset(fe[:, C2 + 8:], 0.0)
                        nc.scalar.activation(fe[:, :C2], h2[:, 0:C2],
                                             mybir.ActivationFunctionType.Copy)
                        nc.vector.tensor_copy(fe[:, C2:C2 + 8].bitcast(F32),
                                              h2[:, C2:C2 + 4])
                        nc.scalar.dma_start(agf_in[base:base + nn, :], fe[:nn, :])
                        sa = l2e.tile([128, 4], F32, tag="sa")
                        nc.vector.tensor_copy(sa[:], h2[:, C2 + 4:C2 + 8])
                        nc.sync.dma_start(d2attn[base:base + nn, :], sa[:nn, :])
                    if after_macro is not None:
                        after_macro(mi)

        def ag_cb(mi):
            if mi == 12:
                nc.gpsimd.collective_compute(
                    "AllGather", mybir.AluOpType.bypass,
                    ins=[agf_in[0:CH0, :]], outs=[t2a[:]],
                    replica_groups=[list(range(NCORES))])

        if stage == 1:
            dummy_out()
        if stage >= 2:
            agg_layer(st1, 1, write_out=(stage == 2),
                      after_macro=ag_cb if stage >= 3 else None)
        if stage >= 3:
            nc.gpsimd.collective_compute(
                "AllGather", mybir.AluOpType.bypass,
                ins=[agf_in[CH0:slab, :]], outs=[t2b[:]],
                replica_groups=[list(range(NCORES))])
        if stage == 3:
            dummy_out()
        if stage >= 4:
            agg_layer(st2, 2)

    nc.compile()
    return nc


# ---------------------------------------------------------------- entry point

def _run(x, edge_index, W1, att_src1, att_dst1, b1, W2, att_src2, att_dst2, b2,
         split=32768):
    x = np.asarray(x, np.float32)
    N, Cin = x.shape
    C1 = W1.shape[1]
    C2 = W2.shape[1]
    slab = N // NCORES

    loops = np.arange(N, dtype=np.int64)
    src = np.concatenate([np.asarray(edge_index[0], np.int64), loops])
    dst = np.concatenate([np.asarray(edge_index[1], np.int64), loops])

    CH0 = 3328
    CH1 = slab - CH0

    def grp1(s):
        return s >= split

    def idx1(s):
        return np.where(s >= split, s - split, s)

    def grp2(s):
        return (s % slab) >= CH0

    def idx2(s):
        r = s // slab
        l = s % slab
        return np.where(l >= CH0, r * CH1 + (l - CH0), r * CH0 + l)

    st1 = _build_structure(src, dst, N, slab, grp1, idx1)
    st2 = _build_structure(src, dst, N, slab, grp2, idx2)
    nc = _build_nc(st1, st2, N, slab, C1, C2, Cin)

    W1e = _fold_ext(np.asarray(W1, np.float32), np.asarray(att_src1, np.float32),
                    np.asarray(att_dst1, np.float32)).astype(bf16)
    W2e = _fold_ext(np.asarray(W2, np.float32), np.asarray(att_src2, np.float32),
                    np.asarray(att_dst2, np.float32)).astype(bf16)
    w2e_arr = np.ascontiguousarray(
        W2e.reshape(C1 // 128, 128, C2 + 2 * HEADS).transpose(1, 0, 2))
    xTb = np.ascontiguousarray(x.T.astype(bf16))
    ident = np.eye(128, dtype=np.float32)

    in_maps = []
    for core in range(NCORES):
        sl = slice(core * slab, (core + 1) * slab)
        in_maps.append({
            "xT": xTb,
            "xoT": np.ascontiguousarray(xTb[:, sl]),
            "w1e": W1e,
            "w2e": w2e_arr,
            "b1": np.asarray(b1, np.float32).reshape(1, C1),
            "b2": np.asarray(b2, np.float32).reshape(1, C2),
            "idf32": ident,
            "IDX1": st1.IDXs[core],
            "SF1": st1.SFs[core],
            "STF1": st1.STFs[core],
            "IDX2": st2.IDXs[core],
            "SF2": st2.SFs[core],
            "STF2": st2.STFs[core],
        })

    res = run_bass_kernel_spmd(nc, in_maps, core_ids=list(range(NCORES)),
                               trace=TRACE)
    if TRACE:
        global LAST_RESULT
        LAST_RESULT = res
    outv = np.concatenate([res.results[c]["out"] for c in range(NCORES)], axis=0)
    return outv.astype(np.float32)


def kernel(x, edge_index, W1, att_src1, att_dst1, b1, W2, att_src2, att_dst2,
           b2):
    return _run(x, edge_index, W1, att_src1, att_dst1, b1,
                W2, att_src2, att_dst2, b2)

